# revision 15
# baseline (speedup 1.0000x reference)
"""CGCNN conv kernel for 8 TRN2 NeuronCores (Bass/Tile).

Strategy (edge-parallel, dst-sharded, row-major):
  z @ W = nf[src] @ W[0:64] + nf[dst] @ W[64:128] + ef @ W[128:160]
  - Host precomputes P_src = nf @ [Wi[:64]|Wu[:64]]  (bf16 [N,128], 256B rows)
                    P_dst = nf @ [Wi[64:128]|Wu[64:128]]
  - Edges sorted by (dst//R, src//CH, src); core c owns dst range
    [c*R,(c+1)*R) so the segment-sum is core-local; src chunks of CH=25000
    keep gather indices in int16.
  - Pass 1 (row-major): non-transposed dma_gather of P rows (512-idx calls
    rotated over 4 SWDGE queues); PE computes ef@W3 into row-major PSUM;
    DVE adds the gathered rows; per-feature BN stats via strided reduces
    accumulated in [128,128] partials; x stored to DRAM bf16 row-major.
  - AllReduce [1,256] edge-BN stats; scale/shift broadcast to [128,128]
    tiles via PE ones-outer-product.
  - Pass 2: reload x; BN applied with broadcast_to DVE ops; ACT sigmoid /
    exp / ln(1+u); msg = gate*sp row-major feeds dma_scatter_add directly
    (occurrence-rank segments into rotating agg buffers, queue-rotated).
  - Phase 3: node-BN stats AllReduce [F,2]; out = softplus(nf + bn(agg))
    feat-major; host transposes back.
"""

import itertools
import math
import sys

import numpy as np

for _p in ("/opt/trn_rl_repo", "/root/.axon_site/_ro/trn_rl_repo"):
    if _p not in sys.path:
        sys.path.append(_p)

import ml_dtypes
from concourse import bacc, bass, mybir
from concourse import tile as ctile
from concourse.bass_utils import run_bass_kernel_spmd
from concourse.masks import make_identity

P = 128
F = 64  # node feature dim; 2F == P
EPS = 1e-5
BF16 = ml_dtypes.bfloat16
NQ = 4  # SWDGE queues
GQ = 512  # indices per gather call

Alu = mybir.AluOpType
Act = mybir.ActivationFunctionType
dt = mybir.dt


def _cfg(N, E, FE, T=2048, ncores=8):
    R = N // ncores
    assert R * ncores == N
    nchunk = max(1, math.ceil(N / 25000))
    CH = math.ceil(N / nchunk)
    assert CH + 1 <= 32768 and R + 1 <= 32768
    r_pad = math.ceil((R + 1) / P) * P
    return dict(
        N=N, E=E, FE=FE, T=T, NC=ncores,
        R=R, NCHUNK=nchunk, CH=CH, R_PAD=r_pad,
    )


def build_graph(cfg):
    NC, T, FE = cfg["NC"], cfg["T"], cfg["FE"]
    CH, NCHUNK, R_PAD = cfg["CH"], cfg["NCHUNK"], cfg["R_PAD"]
    TPC, ETOT = cfg["TPC"], cfg["ETOT"]
    SEGS = list(cfg["SEGS"])
    nseg = len(SEGS)
    NAGG = 8  # rotating scatter-accumulator buffers
    NTILES = NCHUNK * TPC
    NBLK = NTILES // 2
    NGRP = R_PAD // P
    G = T // P  # row-major groups per tile
    B = 2 * T
    inv_e = 1.0 / float(cfg["E"])
    inv_n = 1.0 / float(cfg["N"])
    qc = itertools.count()
    sb = itertools.count()

    nc = bacc.Bacc("TRN2", target_bir_lowering=False, debug=False,
                   num_devices=NC, num_swdge_queues=NQ)

    psrc = [nc.dram_tensor(f"psrc{c}", [CH + 1, P], dt.bfloat16,
                           kind="ExternalInput") for c in range(NCHUNK)]
    pdst = nc.dram_tensor("pdst", [R_PAD, P], dt.bfloat16, kind="ExternalInput")
    eft = nc.dram_tensor("eft", [FE, ETOT], dt.bfloat16, kind="ExternalInput")
    srcidx = nc.dram_tensor("srcidx", [P, ETOT // 16], dt.int16,
                            kind="ExternalInput")
    dstidx = nc.dram_tensor("dstidx", [P, ETOT // 16], dt.int16,
                            kind="ExternalInput")
    nftr = nc.dram_tensor("nftr", [P, (R_PAD // P) * F], dt.float32,
                          kind="ExternalInput")
    w3 = nc.dram_tensor("w3", [FE, P], dt.bfloat16, kind="ExternalInput")
    gvr = nc.dram_tensor("gvr", [1, P], dt.float32, kind="ExternalInput")
    bvr = nc.dram_tensor("bvr", [1, P], dt.float32, kind="ExternalInput")
    gbn = nc.dram_tensor("gbn", [1, F], dt.float32, kind="ExternalInput")
    bbn = nc.dram_tensor("bbn", [1, F], dt.float32, kind="ExternalInput")
    orow = nc.dram_tensor("orow", [P, (R_PAD // P) * F], dt.float32,
                          kind="ExternalOutput")

    xrow = nc.dram_tensor("xrow", [NBLK, P, B], dt.bfloat16, kind="Internal")
    aggd = [nc.dram_tensor(f"aggd{r}", [NGRP, P, F], dt.float32,
                           kind="Internal") for r in range(NAGG)]
    cc1i = nc.dram_tensor("cc1i", [1, 2 * P], dt.float32, kind="Internal")
    cc1o = nc.dram_tensor("cc1o", [1, 2 * P], dt.float32, kind="Internal",
                          addr_space="Shared")
    cc2i = nc.dram_tensor("cc2i", [1, 2 * F], dt.float32, kind="Internal")
    cc2o = nc.dram_tensor("cc2o", [1, 2 * F], dt.float32, kind="Internal",
                          addr_space="Shared")
    rg = [list(range(NC))]

    with ctile.TileContext(nc) as tc:
        with tc.tile_pool(name="const", bufs=1) as cp:
            w3_sb = cp.tile([FE, P], dt.bfloat16)
            nc.sync.dma_start(w3_sb[:], w3.ap())
            gv = cp.tile([1, P], dt.float32)
            nc.sync.dma_start(gv[:], gvr.ap())
            bv = cp.tile([1, P], dt.float32)
            nc.sync.dma_start(bv[:], bvr.ap())
            gbn_sb = cp.tile([1, F], dt.float32)
            nc.sync.dma_start(gbn_sb[:], gbn.ap())
            bbn_sb = cp.tile([1, F], dt.float32)
            nc.sync.dma_start(bbn_sb[:], bbn.ap())
            ones1 = cp.tile([1, P], dt.float32)
            nc.vector.memset(ones1[:], 1.0)
            onesc = cp.tile([P, 1], dt.float32)
            nc.vector.memset(onesc[:], 1.0)

            accs = cp.tile([P, P], dt.float32)
            nc.vector.memset(accs[:], 0.0)
            accq = cp.tile([P, P], dt.float32)
            nc.vector.memset(accq[:], 0.0)
            svbc = cp.tile([P, P], dt.bfloat16)
            bvbc = cp.tile([P, P], dt.bfloat16)

            # zero-fill agg accumulators
            zb = cp.tile([P, 512], dt.float32)
            nc.vector.memset(zb[:], 0.0)
            gper = 512 // F
            for r in range(NAGG):
                for g0 in range(0, NGRP, gper):
                    ng = min(gper, NGRP - g0)
                    nc.sync.dma_start(aggd[r].ap()[g0:g0 + ng, :, :],
                                      zb[:, :ng * F])

            # ---------------- pass 1 (row-major) ----------------
            with tc.tile_pool(name="p1", bufs=4) as p1, \
                 tc.tile_pool(name="p1i", bufs=4) as p1i, \
                 tc.tile_pool(name="ps1", bufs=4, space="PSUM") as ps1:
                for c in range(NCHUNK):
                    for tl in range(TPC):
                        t = c * TPC + tl
                        sidx = p1i.tile([P, T // 16], dt.int16, tag="sidx")
                        nc.sync.dma_start(
                            sidx[:], srcidx.ap()[:, t * (T // 16):
                                                 (t + 1) * (T // 16)])
                        didx = p1i.tile([P, T // 16], dt.int16, tag="didx")
                        nc.sync.dma_start(
                            didx[:], dstidx.ap()[:, t * (T // 16):
                                                 (t + 1) * (T // 16)])
                        srcg = p1.tile([P, G, P], dt.bfloat16, tag="srcg")
                        dstg = p1.tile([P, G, P], dt.bfloat16, tag="dstg")
                        for q in range(T // GQ):
                            gs = slice(q * (GQ // P), (q + 1) * (GQ // P))
                            qi = slice(q * (GQ // 16), (q + 1) * (GQ // 16))
                            nc.gpsimd.dma_gather(
                                srcg[:, gs, :], psrc[c].ap(), sidx[:, qi],
                                GQ, GQ, P, queue_num=next(qc) % NQ)
                            nc.gpsimd.dma_gather(
                                dstg[:, gs, :], pdst.ap(), didx[:, qi],
                                GQ, GQ, P, queue_num=next(qc) % NQ)
                        eftt = p1.tile([FE, T], dt.bfloat16, tag="eftt")
                        nc.sync.dma_start(eftt[:], eft.ap()[:, t * T:(t + 1) * T])

                        xh = p1.tile([P, T], dt.bfloat16, tag="xh")
                        srcf = srcg[:].rearrange("p a b -> p (a b)")
                        dstf = dstg[:].rearrange("p a b -> p (a b)")
                        nc.vector.tensor_tensor(srcf, srcf, dstf, Alu.add)
                        for s in range(T // 512):
                            ps = ps1.tile([P, 512], dt.float32, tag="ps")
                            for g in range(4):
                                col = (s * 4 + g) * P
                                nc.tensor.matmul(
                                    ps[:, g * P:(g + 1) * P],
                                    eftt[:, col:col + P], w3_sb[:],
                                    start=True, stop=True)
                            sl = slice(s * 512, (s + 1) * 512)
                            nc.vector.tensor_tensor(
                                xh[:, sl], ps[:], srcf[:, sl], Alu.add)
                        nc.scalar.dma_start(
                            xrow.ap()[t // 2, :, (t % 2) * T:(t % 2 + 1) * T],
                            xh[:])
                        xsq = p1.tile([P, T], dt.bfloat16, tag="xsq")
                        nc.scalar.activation(xsq[:], xh[:], Act.Square)
                        # contiguous pairwise tree-fold of the 16 row-groups
                        # down to one [P,P] per-feature partial, then fp32
                        # accumulate
                        for tile, acc in ((xh, accs), (xsq, accq)):
                            for w in (8, 4, 2, 1):
                                nc.vector.tensor_tensor(
                                    tile[:, :w * P], tile[:, :w * P],
                                    tile[:, w * P:2 * w * P], Alu.add)
                            nc.vector.tensor_tensor(acc[:], acc[:],
                                                    tile[:, :P], Alu.add)

            # ---------------- edge-BN stats ----------------
            with tc.tile_pool(name="st", bufs=1) as stp, \
                 tc.tile_pool(name="pst", bufs=2, space="PSUM") as pst:
                sps = pst.tile([1, P], dt.float32, tag="sps")
                nc.tensor.matmul(sps[:], onesc[:], accs[:],
                                 start=True, stop=True)
                qps = pst.tile([1, P], dt.float32, tag="qps")
                nc.tensor.matmul(qps[:], onesc[:], accq[:],
                                 start=True, stop=True)
                cst = stp.tile([1, 2 * P], dt.float32)
                nc.vector.tensor_copy(cst[:, 0:P], sps[:])
                nc.vector.tensor_copy(cst[:, P:2 * P], qps[:])
                nc.sync.dma_start(cc1i.ap(), cst[:])
                nc.gpsimd.collective_compute(
                    "AllReduce", Alu.add, replica_groups=rg,
                    ins=[cc1i.ap().opt()], outs=[cc1o.ap().opt()])
                gst = stp.tile([1, 2 * P], dt.float32)
                nc.scalar.dma_start(gst[:], cc1o.ap())

                mu = stp.tile([1, P], dt.float32)
                nc.vector.tensor_scalar(mu[:], gst[:, 0:P], inv_e, None,
                                        Alu.mult)
                veps = stp.tile([1, P], dt.float32)
                musq = stp.tile([1, P], dt.float32)
                nc.vector.tensor_tensor(musq[:], mu[:], mu[:], Alu.mult)
                nc.vector.tensor_scalar(veps[:], gst[:, P:2 * P], inv_e, None,
                                        Alu.mult)
                nc.vector.tensor_tensor(veps[:], veps[:], musq[:],
                                        Alu.subtract)
                nc.vector.tensor_scalar(veps[:], veps[:], EPS, None, Alu.add)
                sdv = stp.tile([1, P], dt.float32)
                nc.scalar.sqrt(sdv[:], veps[:])
                isd = stp.tile([1, P], dt.float32)
                nc.vector.reciprocal(isd[:], sdv[:])
                scl = stp.tile([1, P], dt.float32)
                nc.vector.tensor_tensor(scl[:], gv[:], isd[:], Alu.mult)
                shf = stp.tile([1, P], dt.float32)
                nc.vector.tensor_tensor(shf[:], mu[:], scl[:], Alu.mult)
                nc.vector.tensor_tensor(shf[:], bv[:], shf[:], Alu.subtract)

                bps = pst.tile([P, P], dt.float32, tag="bps")
                nc.tensor.matmul(bps[:], ones1[:], scl[:], start=True,
                                 stop=True)
                nc.vector.tensor_copy(svbc[:], bps[:])
                bps2 = pst.tile([P, P], dt.float32, tag="bps")
                nc.tensor.matmul(bps2[:], ones1[:], shf[:], start=True,
                                 stop=True)
                nc.vector.tensor_copy(bvbc[:], bps2[:])

            # ---------------- pass 2 ----------------
            GB = B // P  # 32 row-groups per block
            soff = np.cumsum([0] + SEGS)
            PAIR = 2
            with tc.tile_pool(name="p2", bufs=2 * PAIR + 1) as p2, \
                 tc.tile_pool(name="p2i", bufs=6) as p2i:
                for b0 in range(0, NBLK, PAIR):
                    blks = range(b0, min(b0 + PAIR, NBLK))
                    xns, gates, us, sps, msgs = {}, {}, {}, {}, {}
                    for b in blks:
                        xn = p2.tile([P, GB, P], dt.bfloat16, tag="xn")
                        nc.sync.dma_start(
                            xn[:],
                            xrow.ap()[b].rearrange("p (a b) -> p a b", b=P))
                        nc.vector.tensor_tensor(
                            xn[:], xn[:],
                            svbc[:, None, :].broadcast_to([P, GB, P]),
                            Alu.mult)
                        nc.vector.tensor_tensor(
                            xn[:], xn[:],
                            bvbc[:, None, :].broadcast_to([P, GB, P]),
                            Alu.add)
                        xns[b] = xn
                    for b in blks:
                        gate = p2.tile([P, GB, F], dt.bfloat16, tag="gate")
                        nc.scalar.activation(gate[:], xns[b][:, :, 0:F],
                                             Act.Sigmoid)
                        gates[b] = gate
                    for b in blks:
                        u = p2.tile([P, GB, F], dt.bfloat16, tag="u")
                        nc.scalar.activation(u[:], xns[b][:, :, F:P], Act.Exp)
                        us[b] = u
                    for b in blks:
                        sp = p2.tile([P, GB, F], dt.bfloat16, tag="sp")
                        nc.scalar.activation(sp[:], us[b][:], Act.Ln,
                                             bias=1.0, scale=1.0)
                        sps[b] = sp
                    for b in blks:
                        msg = p2.tile([P, GB, F], dt.float32, tag="msg")
                        nc.vector.tensor_tensor(msg[:], gates[b][:],
                                                sps[b][:], Alu.mult)
                        msgs[b] = msg
                    for b in blks:
                        didx2 = p2i.tile([P, B // 16], dt.int16, tag="didx2")
                        nc.sync.dma_start(
                            didx2[:],
                            dstidx.ap()[:, b * (B // 16):(b + 1) * (B // 16)])
                        for r, sr in enumerate(SEGS):
                            ri = next(sb) % NAGG
                            o0 = int(soff[r])
                            nc.gpsimd.dma_scatter_add(
                                aggd[ri].ap().flatten_outer_dims(),
                                msgs[b][:, o0 // P:(o0 + sr) // P, :],
                                didx2[:, o0 // 16:(o0 + sr) // 16],
                                sr, sr, F, queue_num=next(qc) % NQ)

            # ---------------- phase 3 (row-major) ----------------
            with tc.tile_pool(name="p3", bufs=1) as p3, \
                 tc.tile_pool(name="p3c", bufs=4) as p3c, \
                 tc.tile_pool(name="ps3", bufs=1, space="PSUM") as ps3:
                Rr = cfg["R"]
                MG = 14  # node groups per merge chunk
                aggm = p3.tile([P, NGRP, F], dt.float32)
                for q0 in range(0, NGRP, MG):
                    nq_ = min(MG, NGRP - q0)
                    first = True
                    for r in range(NAGG):
                        at = p3c.tile([P, MG, F], dt.float32,
                                      tag=f"at{r % 4}")
                        nc.sync.dma_start(
                            at[:, :nq_, :],
                            aggd[r].ap()[q0:q0 + nq_].rearrange(
                                "g p d -> p g d"))
                        if first:
                            nc.vector.tensor_copy(
                                aggm[:, q0:q0 + nq_, :], at[:, :nq_, :])
                            first = False
                        else:
                            nc.vector.tensor_tensor(
                                aggm[:, q0:q0 + nq_, :],
                                aggm[:, q0:q0 + nq_, :], at[:, :nq_, :],
                                Alu.add)
                # scatter pads all target node row R; rows R+1.. stay
                # zero from the zero-fill. Zero row R via partition-offset
                # DMA (DVE cannot address partition 84).
                lastg = Rr // P
                p0pad = Rr - lastg * P
                assert lastg == NGRP - 1
                nc.sync.dma_start(aggm[p0pad:p0pad + 1, lastg, :],
                                  zb[0:1, :F])

                # node-BN stats: per-feature sums over [p, g] cells
                sacc = p3.tile([P, F], dt.float32)
                nc.vector.tensor_reduce(
                    sacc[:], aggm[:].rearrange("p g f -> p f g"),
                    mybir.AxisListType.X, Alu.add)
                sqm = p3.tile([P, NGRP, F], dt.bfloat16)
                nc.scalar.activation(sqm[:], aggm[:], Act.Square)
                qacc = p3.tile([P, F], dt.float32)
                nc.vector.tensor_reduce(
                    qacc[:], sqm[:].rearrange("p g f -> p f g"),
                    mybir.AxisListType.X, Alu.add)
                s2ps = ps3.tile([1, F], dt.float32, tag="s2ps")
                nc.tensor.matmul(s2ps[:], onesc[:], sacc[:], start=True,
                                 stop=True)
                q2ps = ps3.tile([1, F], dt.float32, tag="q2ps")
                nc.tensor.matmul(q2ps[:], onesc[:], qacc[:], start=True,
                                 stop=True)
                c2st = p3.tile([1, 2 * F], dt.float32)
                nc.vector.tensor_copy(c2st[:, 0:F], s2ps[:])
                nc.vector.tensor_copy(c2st[:, F:2 * F], q2ps[:])
                nc.sync.dma_start(cc2i.ap(), c2st[:])
                nc.gpsimd.collective_compute(
                    "AllReduce", Alu.add, replica_groups=rg,
                    ins=[cc2i.ap().opt()], outs=[cc2o.ap().opt()])
                gs2 = p3.tile([1, 2 * F], dt.float32)
                nc.scalar.dma_start(gs2[:], cc2o.ap())

                mu2 = p3.tile([1, F], dt.float32)
                nc.vector.tensor_scalar(mu2[:], gs2[:, 0:F], inv_n, None,
                                        Alu.mult)
                ve2 = p3.tile([1, F], dt.float32)
                ms2 = p3.tile([1, F], dt.float32)
                nc.vector.tensor_tensor(ms2[:], mu2[:], mu2[:], Alu.mult)
                nc.vector.tensor_scalar(ve2[:], gs2[:, F:2 * F], inv_n, None,
                                        Alu.mult)
                nc.vector.tensor_tensor(ve2[:], ve2[:], ms2[:], Alu.subtract)
                nc.vector.tensor_scalar(ve2[:], ve2[:], EPS, None, Alu.add)
                sd2 = p3.tile([1, F], dt.float32)
                nc.scalar.sqrt(sd2[:], ve2[:])
                is2 = p3.tile([1, F], dt.float32)
                nc.vector.reciprocal(is2[:], sd2[:])
                sc2 = p3.tile([1, F], dt.float32)
                nc.vector.tensor_tensor(sc2[:], gbn_sb[:], is2[:], Alu.mult)
                sh2 = p3.tile([1, F], dt.float32)
                nc.vector.tensor_tensor(sh2[:], mu2[:], sc2[:], Alu.mult)
                nc.vector.tensor_tensor(sh2[:], bbn_sb[:], sh2[:],
                                        Alu.subtract)
                b2ps = ps3.tile([P, F], dt.float32, tag="b2ps")
                nc.tensor.matmul(b2ps[:], ones1[:], sc2[:], start=True,
                                 stop=True)
                sv2 = p3.tile([P, F], dt.float32)
                nc.vector.tensor_copy(sv2[:], b2ps[:])
                b2ps2 = ps3.tile([P, F], dt.float32, tag="b2ps")
                nc.tensor.matmul(b2ps2[:], ones1[:], sh2[:], start=True,
                                 stop=True)
                bv2 = p3.tile([P, F], dt.float32)
                nc.vector.tensor_copy(bv2[:], b2ps2[:])

                # out = softplus(nf + bn(agg)), all row-major
                nfr = p3.tile([P, NGRP, F], dt.float32)
                nc.sync.dma_start(
                    nfr[:], nftr.ap().rearrange("p (g f) -> p g f", f=F))
                nc.vector.tensor_tensor(
                    aggm[:], aggm[:],
                    sv2[:, None, :].broadcast_to([P, NGRP, F]), Alu.mult)
                nc.vector.tensor_tensor(
                    aggm[:], aggm[:],
                    bv2[:, None, :].broadcast_to([P, NGRP, F]), Alu.add)
                nc.vector.tensor_tensor(aggm[:], aggm[:], nfr[:], Alu.add)
                u3 = p3.tile([P, NGRP, F], dt.float32)
                nc.scalar.activation(u3[:], aggm[:], Act.Exp)
                nc.scalar.activation(aggm[:], u3[:], Act.Ln, bias=1.0,
                                     scale=1.0)
                nc.sync.dma_start(
                    orow.ap().rearrange("p (g f) -> p g f", f=F), aggm[:])

    nc.compile()
    return nc


_CACHE = {}


def _prep(inputs, T=2048):
    nf = np.ascontiguousarray(np.asarray(inputs["node_feats"], np.float32))
    ef = np.ascontiguousarray(np.asarray(inputs["edge_feats"], np.float32))
    src = np.asarray(inputs["src"], np.int64)
    dst = np.asarray(inputs["dst"], np.int64)
    Wi = np.asarray(inputs["W_int"], np.float32)
    Wu = np.asarray(inputs["W_upd"], np.float32)
    N, Fn = nf.shape
    E, FE = ef.shape
    assert Fn == F
    cfg = _cfg(N, E, FE, T=T)
    NCh, CH, R, NCc = cfg["NCHUNK"], cfg["CH"], cfg["R"], cfg["NC"]

    # b_int/b_upd are dropped: a constant bias shifts mean equally and
    # cancels inside BatchNorm.
    Psrc = (nf @ np.concatenate([Wi[:F], Wu[:F]], axis=1)).astype(BF16)
    Pdst = (nf @ np.concatenate([Wi[F:2 * F], Wu[F:2 * F]], axis=1)).astype(BF16)
    W3 = np.concatenate([Wi[2 * F:], Wu[2 * F:]], axis=1).astype(BF16)

    core = dst // R
    chunk = src // CH
    key = core * NCh + chunk
    order = np.lexsort((src, key))
    counts = np.bincount(key, minlength=NCc * NCh)
    gstart = np.zeros(NCc * NCh + 1, np.int64)
    np.cumsum(counts, out=gstart[1:])

    # ---- occurrence-rank block filling -------------------------------
    # dma_scatter_add cannot accumulate duplicate indices within one call
    # (the CCE read-modify-write races between M2S reads and S2M writes),
    # so each block of B edges is split into rank segments: seg r holds
    # the (r+1)-th occurrences of dst values within the block, each seg
    # internally dst-unique, scattered by its own call into its own agg
    # buffer. Calls on one buffer are WAW-serialized by Tile.
    B = 2 * T

    def occ_ranks(d):
        o = np.argsort(d, kind="stable")
        sd = d[o]
        newrun = np.r_[True, sd[1:] != sd[:-1]]
        ii = np.arange(len(d))
        runstart = np.maximum.accumulate(np.where(newrun, ii, 0))
        occ = np.empty(len(d), np.int64)
        occ[o] = ii - runstart
        return occ

    prof = np.zeros(256, np.float64)
    npool = 0
    for g in range(NCc * NCh):
        dd = dst[order[gstart[g]:gstart[g + 1]]]
        for p0 in range(0, len(dd), B):
            oc = occ_ranks(dd[p0:p0 + B])
            bc = np.bincount(oc, minlength=256)[:256]
            prof += bc
            npool += 1
    prof /= max(npool, 1)
    segs = []
    for r in range(1, 256):
        if prof[r] < 24:
            break
        s_r = max(128, int(round(prof[r] / 128)) * 128)
        if sum(segs) + s_r > B - 512:
            break
        segs.append(s_r)
    SEGS = [B - sum(segs)] + segs
    cfg["SEGS"] = tuple(SEGS)
    soff = np.cumsum([0] + SEGS)

    def fill_chunk(eidx):
        blocks = []
        carry = np.empty(0, np.int64)
        ptr = 0
        n = len(eidx)
        while ptr < n or len(carry):
            take = min(B - len(carry), n - ptr)
            pool = np.concatenate([carry, eidx[ptr:ptr + take]])
            ptr += take
            oc = occ_ranks(dst[pool])
            slots = np.full(B, -1, np.int64)
            used = np.zeros(len(pool), bool)
            for r, sr in enumerate(SEGS):
                cand = np.flatnonzero(oc == r)[:sr]
                slots[soff[r]:soff[r] + len(cand)] = pool[cand]
                used[cand] = True
            carry = pool[~used]
            blocks.append(slots)
        return blocks

    core_blocks = []
    nbc = 0
    for c in range(NCc):
        per_chunk = []
        for k in range(NCh):
            g = c * NCh + k
            blks = fill_chunk(order[gstart[g]:gstart[g + 1]])
            nbc = max(nbc, len(blks))
            per_chunk.append(blks)
        core_blocks.append(per_chunk)

    tpc = 2 * nbc
    KT = tpc * T
    ETOT = NCh * KT
    cfg["TPC"], cfg["ETOT"] = tpc, ETOT

    in_maps = []
    psrc_arrs = []
    for k in range(NCh):
        tab = np.zeros((CH + 1, P), BF16)
        hi = min((k + 1) * CH, N)
        tab[: hi - k * CH] = Psrc[k * CH: hi]
        psrc_arrs.append(tab)
    gvec = np.concatenate([np.asarray(inputs["g_int"], np.float32),
                           np.asarray(inputs["g_upd"], np.float32)])[None, :]
    bvec = np.concatenate([np.asarray(inputs["be_int"], np.float32),
                           np.asarray(inputs["be_upd"], np.float32)])[None, :]
    gbn = np.asarray(inputs["g_bn"], np.float32)[None, :]
    bbn = np.asarray(inputs["be_bn"], np.float32)[None, :]

    for c in range(NCc):
        src_l = np.full(ETOT, CH, np.int16)
        dst_l = np.full(ETOT, R, np.int16)
        eftc = np.zeros((FE, ETOT), BF16)
        for k in range(NCh):
            slotc = np.full(KT, -1, np.int64)
            blks = core_blocks[c][k]
            for bi, blk in enumerate(blks):
                slotc[bi * B:(bi + 1) * B] = blk
            mask = slotc >= 0
            sel = slotc[mask]
            pos = np.flatnonzero(mask) + k * KT
            src_l[pos] = (src[sel] - k * CH).astype(np.int16)
            dst_l[pos] = (dst[sel] - c * R).astype(np.int16)
            eftc[:, pos] = ef[sel].T
        pd = np.zeros((cfg["R_PAD"], P), BF16)
        pd[:R] = Pdst[c * R:(c + 1) * R]
        nfp = np.zeros((cfg["R_PAD"], F), np.float32)
        nfp[:R] = nf[c * R:(c + 1) * R]
        ngrp = cfg["R_PAD"] // P
        nftc = np.ascontiguousarray(
            nfp.reshape(ngrp, P, F).transpose(1, 0, 2).reshape(P, ngrp * F))
        m = {
            "pdst": pd,
            "eft": eftc,
            "srcidx": np.ascontiguousarray(
                np.tile(src_l.reshape(ETOT // 16, 16).T, (P // 16, 1))),
            "dstidx": np.ascontiguousarray(
                np.tile(dst_l.reshape(ETOT // 16, 16).T, (P // 16, 1))),
            "nftr": nftc,
            "w3": W3,
            "gvr": gvec, "bvr": bvec, "gbn": gbn, "bbn": bbn,
        }
        for k in range(NCh):
            m[f"psrc{k}"] = psrc_arrs[k]
        in_maps.append(m)
    return cfg, in_maps


def _run(inputs, T=2048, trace=False):
    cfg, in_maps = _prep(inputs, T=T)
    ck = (cfg["N"], cfg["E"], cfg["FE"], cfg["T"], cfg["TPC"], cfg["SEGS"])
    if ck not in _CACHE:
        _CACHE[ck] = build_graph(cfg)
    nc = _CACHE[ck]
    res = run_bass_kernel_spmd(nc, in_maps, core_ids=list(range(cfg["NC"])),
                               trace=trace)
    R = cfg["R"]
    ngrp = cfg["R_PAD"] // P
    outs = []
    for c in range(cfg["NC"]):
        o = np.asarray(res.results[c]["orow"]).reshape(P, ngrp, F)
        outs.append(o.transpose(1, 0, 2).reshape(ngrp * P, F)[:R])
    out = np.concatenate(outs, axis=0)
    return np.ascontiguousarray(out, dtype=np.float32), res


def kernel(**inputs) -> np.ndarray:
    out, _ = _run(inputs)
    return out


# revision 16
# speedup vs baseline: 1.0162x; 1.0162x over previous
"""CGCNN conv kernel for 8 TRN2 NeuronCores (Bass/Tile).

Strategy (edge-parallel, dst-sharded, row-major):
  z @ W = nf[src] @ W[0:64] + nf[dst] @ W[64:128] + ef @ W[128:160]
  - Host precomputes P_src = nf @ [Wi[:64]|Wu[:64]]  (bf16 [N,128], 256B rows)
                    P_dst = nf @ [Wi[64:128]|Wu[64:128]]
  - Edges sorted by (dst//R, src//CH, src); core c owns dst range
    [c*R,(c+1)*R) so the segment-sum is core-local; src chunks of CH=25000
    keep gather indices in int16.
  - Pass 1 (row-major): non-transposed dma_gather of P rows (512-idx calls
    rotated over 4 SWDGE queues); PE computes ef@W3 into row-major PSUM;
    DVE adds the gathered rows; per-feature BN stats via strided reduces
    accumulated in [128,128] partials; x stored to DRAM bf16 row-major.
  - AllReduce [1,256] edge-BN stats; scale/shift broadcast to [128,128]
    tiles via PE ones-outer-product.
  - Pass 2: reload x; BN applied with broadcast_to DVE ops; ACT sigmoid /
    exp / ln(1+u); msg = gate*sp row-major feeds dma_scatter_add directly
    (occurrence-rank segments into rotating agg buffers, queue-rotated).
  - Phase 3: node-BN stats AllReduce [F,2]; out = softplus(nf + bn(agg))
    feat-major; host transposes back.
"""

import itertools
import math
import sys

import numpy as np

for _p in ("/opt/trn_rl_repo", "/root/.axon_site/_ro/trn_rl_repo"):
    if _p not in sys.path:
        sys.path.append(_p)

import ml_dtypes
from concourse import bacc, bass, mybir
from concourse import tile as ctile
from concourse.bass_utils import run_bass_kernel_spmd
from concourse.masks import make_identity

P = 128
F = 64  # node feature dim; 2F == P
EPS = 1e-5
BF16 = ml_dtypes.bfloat16
NQ = 4  # SWDGE queues
GQ = 512  # indices per gather call

Alu = mybir.AluOpType
Act = mybir.ActivationFunctionType
dt = mybir.dt


def _cfg(N, E, FE, T=2048, ncores=8):
    R = N // ncores
    assert R * ncores == N
    nchunk = max(1, math.ceil(N / 25000))
    CH = math.ceil(N / nchunk)
    assert CH + 1 <= 32768 and R + 1 <= 32768
    r_pad = math.ceil((R + 1) / P) * P
    return dict(
        N=N, E=E, FE=FE, T=T, NC=ncores,
        R=R, NCHUNK=nchunk, CH=CH, R_PAD=r_pad,
    )


def build_graph(cfg):
    NC, T, FE = cfg["NC"], cfg["T"], cfg["FE"]
    CH, NCHUNK, R_PAD = cfg["CH"], cfg["NCHUNK"], cfg["R_PAD"]
    TPC, ETOT = cfg["TPC"], cfg["ETOT"]
    SEGS = list(cfg["SEGS"])
    nseg = len(SEGS)
    NAGG = 8  # rotating scatter-accumulator buffers
    NTILES = NCHUNK * TPC
    NBLK = NTILES // 2
    NGRP = R_PAD // P
    G = T // P  # row-major groups per tile
    B = 2 * T
    inv_e = 1.0 / float(cfg["E"])
    inv_n = 1.0 / float(cfg["N"])
    qc = itertools.count()
    sb = itertools.count()

    nc = bacc.Bacc("TRN2", target_bir_lowering=False, debug=False,
                   num_devices=NC, num_swdge_queues=NQ)

    psrc = [nc.dram_tensor(f"psrc{c}", [CH + 1, P], dt.bfloat16,
                           kind="ExternalInput") for c in range(NCHUNK)]
    pdst = nc.dram_tensor("pdst", [R_PAD, P], dt.bfloat16, kind="ExternalInput")
    eft = nc.dram_tensor("eft", [FE, ETOT], dt.bfloat16, kind="ExternalInput")
    srcidx = nc.dram_tensor("srcidx", [P, ETOT // 16], dt.int16,
                            kind="ExternalInput")
    dstidx = nc.dram_tensor("dstidx", [P, ETOT // 16], dt.int16,
                            kind="ExternalInput")
    nftr = nc.dram_tensor("nftr", [P, (R_PAD // P) * F], dt.float32,
                          kind="ExternalInput")
    w3 = nc.dram_tensor("w3", [FE, P], dt.bfloat16, kind="ExternalInput")
    gvr = nc.dram_tensor("gvr", [1, P], dt.float32, kind="ExternalInput")
    bvr = nc.dram_tensor("bvr", [1, P], dt.float32, kind="ExternalInput")
    gbn = nc.dram_tensor("gbn", [1, F], dt.float32, kind="ExternalInput")
    bbn = nc.dram_tensor("bbn", [1, F], dt.float32, kind="ExternalInput")
    orow = nc.dram_tensor("orow", [P, (R_PAD // P) * F], dt.float32,
                          kind="ExternalOutput")

    xrow = nc.dram_tensor("xrow", [NBLK, P, B], dt.bfloat16, kind="Internal")
    aggd = [nc.dram_tensor(f"aggd{r}", [NGRP, P, F], dt.float32,
                           kind="Internal") for r in range(NAGG)]
    cc1i = nc.dram_tensor("cc1i", [1, 2 * P], dt.float32, kind="Internal")
    cc1o = nc.dram_tensor("cc1o", [1, 2 * P], dt.float32, kind="Internal",
                          addr_space="Shared")
    cc2i = nc.dram_tensor("cc2i", [1, 2 * F], dt.float32, kind="Internal")
    cc2o = nc.dram_tensor("cc2o", [1, 2 * F], dt.float32, kind="Internal",
                          addr_space="Shared")
    rg = [list(range(NC))]

    with ctile.TileContext(nc) as tc:
        with tc.tile_pool(name="const", bufs=1) as cp:
            w3_sb = cp.tile([FE, P], dt.bfloat16)
            nc.sync.dma_start(w3_sb[:], w3.ap())
            gv = cp.tile([1, P], dt.float32)
            nc.sync.dma_start(gv[:], gvr.ap())
            bv = cp.tile([1, P], dt.float32)
            nc.sync.dma_start(bv[:], bvr.ap())
            gbn_sb = cp.tile([1, F], dt.float32)
            nc.sync.dma_start(gbn_sb[:], gbn.ap())
            bbn_sb = cp.tile([1, F], dt.float32)
            nc.sync.dma_start(bbn_sb[:], bbn.ap())
            ones1 = cp.tile([1, P], dt.float32)
            nc.vector.memset(ones1[:], 1.0)
            onesc = cp.tile([P, 1], dt.float32)
            nc.vector.memset(onesc[:], 1.0)

            accs = cp.tile([P, P], dt.float32)
            nc.vector.memset(accs[:], 0.0)
            accq = cp.tile([P, P], dt.float32)
            nc.vector.memset(accq[:], 0.0)
            svbc = cp.tile([P, P], dt.bfloat16)
            bvbc = cp.tile([P, P], dt.bfloat16)

            # zero-fill agg accumulators
            zb = cp.tile([P, 512], dt.float32)
            nc.vector.memset(zb[:], 0.0)
            gper = 512 // F
            for r in range(NAGG):
                for g0 in range(0, NGRP, gper):
                    ng = min(gper, NGRP - g0)
                    nc.sync.dma_start(aggd[r].ap()[g0:g0 + ng, :, :],
                                      zb[:, :ng * F])

            # ---------------- pass 1 (row-major) ----------------
            with tc.tile_pool(name="p1", bufs=4) as p1, \
                 tc.tile_pool(name="p1i", bufs=4) as p1i, \
                 tc.tile_pool(name="ps1", bufs=4, space="PSUM") as ps1:
                for c in range(NCHUNK):
                    for tl in range(TPC):
                        t = c * TPC + tl
                        sidx = p1i.tile([P, T // 16], dt.int16, tag="sidx")
                        nc.sync.dma_start(
                            sidx[:], srcidx.ap()[:, t * (T // 16):
                                                 (t + 1) * (T // 16)])
                        didx = p1i.tile([P, T // 16], dt.int16, tag="didx")
                        nc.sync.dma_start(
                            didx[:], dstidx.ap()[:, t * (T // 16):
                                                 (t + 1) * (T // 16)])
                        srcg = p1.tile([P, G, P], dt.bfloat16, tag="srcg")
                        dstg = p1.tile([P, G, P], dt.bfloat16, tag="dstg")
                        for q in range(T // GQ):
                            gs = slice(q * (GQ // P), (q + 1) * (GQ // P))
                            qi = slice(q * (GQ // 16), (q + 1) * (GQ // 16))
                            nc.gpsimd.dma_gather(
                                srcg[:, gs, :], psrc[c].ap(), sidx[:, qi],
                                GQ, GQ, P, queue_num=next(qc) % NQ)
                            nc.gpsimd.dma_gather(
                                dstg[:, gs, :], pdst.ap(), didx[:, qi],
                                GQ, GQ, P, queue_num=next(qc) % NQ)
                        eftt = p1.tile([FE, T], dt.bfloat16, tag="eftt")
                        nc.sync.dma_start(eftt[:], eft.ap()[:, t * T:(t + 1) * T])

                        xh = p1.tile([P, T], dt.bfloat16, tag="xh")
                        srcf = srcg[:].rearrange("p a b -> p (a b)")
                        dstf = dstg[:].rearrange("p a b -> p (a b)")
                        for s in range(T // 512):
                            ps = ps1.tile([P, 512], dt.float32, tag="ps")
                            for g in range(4):
                                col = (s * 4 + g) * P
                                nc.tensor.matmul(
                                    ps[:, g * P:(g + 1) * P],
                                    eftt[:, col:col + P], w3_sb[:],
                                    start=True, stop=True)
                            sl = slice(s * 512, (s + 1) * 512)
                            nc.vector.tensor_tensor(
                                xh[:, sl], ps[:], srcf[:, sl], Alu.add)
                            nc.vector.tensor_tensor(
                                xh[:, sl], xh[:, sl], dstf[:, sl], Alu.add)
                        nc.scalar.dma_start(
                            xrow.ap()[t // 2, :, (t % 2) * T:(t % 2 + 1) * T],
                            xh[:])
                        xsq = p1.tile([P, T], dt.bfloat16, tag="xsq")
                        nc.scalar.activation(xsq[:], xh[:], Act.Square)
                        # contiguous pairwise tree-fold of the 16 row-groups
                        # down to one [P,P] per-feature partial, then fp32
                        # accumulate
                        for tile, acc in ((xh, accs), (xsq, accq)):
                            for w in (8, 4, 2, 1):
                                nc.vector.tensor_tensor(
                                    tile[:, :w * P], tile[:, :w * P],
                                    tile[:, w * P:2 * w * P], Alu.add)
                            nc.vector.tensor_tensor(acc[:], acc[:],
                                                    tile[:, :P], Alu.add)

            # ---------------- edge-BN stats ----------------
            with tc.tile_pool(name="st", bufs=1) as stp, \
                 tc.tile_pool(name="pst", bufs=2, space="PSUM") as pst:
                sps = pst.tile([1, P], dt.float32, tag="sps")
                nc.tensor.matmul(sps[:], onesc[:], accs[:],
                                 start=True, stop=True)
                qps = pst.tile([1, P], dt.float32, tag="qps")
                nc.tensor.matmul(qps[:], onesc[:], accq[:],
                                 start=True, stop=True)
                cst = stp.tile([1, 2 * P], dt.float32)
                nc.vector.tensor_copy(cst[:, 0:P], sps[:])
                nc.vector.tensor_copy(cst[:, P:2 * P], qps[:])
                nc.sync.dma_start(cc1i.ap(), cst[:])
                nc.gpsimd.collective_compute(
                    "AllReduce", Alu.add, replica_groups=rg,
                    ins=[cc1i.ap().opt()], outs=[cc1o.ap().opt()])
                gst = stp.tile([1, 2 * P], dt.float32)
                nc.scalar.dma_start(gst[:], cc1o.ap())

                mu = stp.tile([1, P], dt.float32)
                nc.vector.tensor_scalar(mu[:], gst[:, 0:P], inv_e, None,
                                        Alu.mult)
                veps = stp.tile([1, P], dt.float32)
                musq = stp.tile([1, P], dt.float32)
                nc.vector.tensor_tensor(musq[:], mu[:], mu[:], Alu.mult)
                nc.vector.tensor_scalar(veps[:], gst[:, P:2 * P], inv_e, None,
                                        Alu.mult)
                nc.vector.tensor_tensor(veps[:], veps[:], musq[:],
                                        Alu.subtract)
                nc.vector.tensor_scalar(veps[:], veps[:], EPS, None, Alu.add)
                sdv = stp.tile([1, P], dt.float32)
                nc.scalar.sqrt(sdv[:], veps[:])
                isd = stp.tile([1, P], dt.float32)
                nc.vector.reciprocal(isd[:], sdv[:])
                scl = stp.tile([1, P], dt.float32)
                nc.vector.tensor_tensor(scl[:], gv[:], isd[:], Alu.mult)
                shf = stp.tile([1, P], dt.float32)
                nc.vector.tensor_tensor(shf[:], mu[:], scl[:], Alu.mult)
                nc.vector.tensor_tensor(shf[:], bv[:], shf[:], Alu.subtract)

                bps = pst.tile([P, P], dt.float32, tag="bps")
                nc.tensor.matmul(bps[:], ones1[:], scl[:], start=True,
                                 stop=True)
                nc.vector.tensor_copy(svbc[:], bps[:])
                bps2 = pst.tile([P, P], dt.float32, tag="bps")
                nc.tensor.matmul(bps2[:], ones1[:], shf[:], start=True,
                                 stop=True)
                nc.vector.tensor_copy(bvbc[:], bps2[:])

            # ---------------- pass 2 ----------------
            GB = B // P  # 32 row-groups per block
            soff = np.cumsum([0] + SEGS)
            PAIR = 2
            with tc.tile_pool(name="p2", bufs=2 * PAIR + 1) as p2, \
                 tc.tile_pool(name="p2i", bufs=6) as p2i:
                for b0 in range(0, NBLK, PAIR):
                    blks = range(b0, min(b0 + PAIR, NBLK))
                    xns, gates, us, sps, msgs = {}, {}, {}, {}, {}
                    for b in blks:
                        xi = p2.tile([P, GB, P], dt.bfloat16, tag="xi")
                        nc.sync.dma_start(
                            xi[:],
                            xrow.ap()[b].rearrange("p (a b) -> p a b", b=P))
                        xn = p2.tile([P, GB, P], dt.bfloat16, tag="xn")
                        nc.vector.tensor_tensor(
                            xn[:], xi[:],
                            svbc[:, None, :].broadcast_to([P, GB, P]),
                            Alu.mult)
                        nc.vector.tensor_tensor(
                            xn[:], xn[:],
                            bvbc[:, None, :].broadcast_to([P, GB, P]),
                            Alu.add)
                        xns[b] = xn
                    for b in blks:
                        gate = p2.tile([P, GB, F], dt.bfloat16, tag="gate")
                        nc.scalar.activation(gate[:], xns[b][:, :, 0:F],
                                             Act.Sigmoid)
                        gates[b] = gate
                    for b in blks:
                        u = p2.tile([P, GB, F], dt.bfloat16, tag="u")
                        nc.scalar.activation(u[:], xns[b][:, :, F:P], Act.Exp)
                        us[b] = u
                    for b in blks:
                        sp = p2.tile([P, GB, F], dt.bfloat16, tag="sp")
                        nc.scalar.activation(sp[:], us[b][:], Act.Ln,
                                             bias=1.0, scale=1.0)
                        sps[b] = sp
                    for b in blks:
                        msg = p2.tile([P, GB, F], dt.float32, tag="msg")
                        nc.vector.tensor_tensor(msg[:], gates[b][:],
                                                sps[b][:], Alu.mult)
                        msgs[b] = msg
                    for b in blks:
                        didx2 = p2i.tile([P, B // 16], dt.int16, tag="didx2")
                        nc.sync.dma_start(
                            didx2[:],
                            dstidx.ap()[:, b * (B // 16):(b + 1) * (B // 16)])
                        for r, sr in enumerate(SEGS):
                            ri = next(sb) % NAGG
                            o0 = int(soff[r])
                            nc.gpsimd.dma_scatter_add(
                                aggd[ri].ap().flatten_outer_dims(),
                                msgs[b][:, o0 // P:(o0 + sr) // P, :],
                                didx2[:, o0 // 16:(o0 + sr) // 16],
                                sr, sr, F, queue_num=next(qc) % NQ)

            # ---------------- phase 3 (row-major) ----------------
            with tc.tile_pool(name="p3", bufs=1) as p3, \
                 tc.tile_pool(name="p3c", bufs=4) as p3c, \
                 tc.tile_pool(name="ps3", bufs=1, space="PSUM") as ps3:
                Rr = cfg["R"]
                MG = 14  # node groups per merge chunk
                aggm = p3.tile([P, NGRP, F], dt.float32)
                for q0 in range(0, NGRP, MG):
                    nq_ = min(MG, NGRP - q0)
                    first = True
                    for r in range(NAGG):
                        at = p3c.tile([P, MG, F], dt.float32,
                                      tag=f"at{r % 4}")
                        nc.sync.dma_start(
                            at[:, :nq_, :],
                            aggd[r].ap()[q0:q0 + nq_].rearrange(
                                "g p d -> p g d"))
                        if first:
                            nc.vector.tensor_copy(
                                aggm[:, q0:q0 + nq_, :], at[:, :nq_, :])
                            first = False
                        else:
                            nc.vector.tensor_tensor(
                                aggm[:, q0:q0 + nq_, :],
                                aggm[:, q0:q0 + nq_, :], at[:, :nq_, :],
                                Alu.add)
                # scatter pads all target node row R; rows R+1.. stay
                # zero from the zero-fill. Zero row R via partition-offset
                # DMA (DVE cannot address partition 84).
                lastg = Rr // P
                p0pad = Rr - lastg * P
                assert lastg == NGRP - 1
                nc.sync.dma_start(aggm[p0pad:p0pad + 1, lastg, :],
                                  zb[0:1, :F])

                # node-BN stats: per-feature sums over [p, g] cells
                sacc = p3.tile([P, F], dt.float32)
                nc.vector.tensor_reduce(
                    sacc[:], aggm[:].rearrange("p g f -> p f g"),
                    mybir.AxisListType.X, Alu.add)
                sqm = p3.tile([P, NGRP, F], dt.bfloat16)
                nc.scalar.activation(sqm[:], aggm[:], Act.Square)
                qacc = p3.tile([P, F], dt.float32)
                nc.vector.tensor_reduce(
                    qacc[:], sqm[:].rearrange("p g f -> p f g"),
                    mybir.AxisListType.X, Alu.add)
                s2ps = ps3.tile([1, F], dt.float32, tag="s2ps")
                nc.tensor.matmul(s2ps[:], onesc[:], sacc[:], start=True,
                                 stop=True)
                q2ps = ps3.tile([1, F], dt.float32, tag="q2ps")
                nc.tensor.matmul(q2ps[:], onesc[:], qacc[:], start=True,
                                 stop=True)
                c2st = p3.tile([1, 2 * F], dt.float32)
                nc.vector.tensor_copy(c2st[:, 0:F], s2ps[:])
                nc.vector.tensor_copy(c2st[:, F:2 * F], q2ps[:])
                nc.sync.dma_start(cc2i.ap(), c2st[:])
                nc.gpsimd.collective_compute(
                    "AllReduce", Alu.add, replica_groups=rg,
                    ins=[cc2i.ap().opt()], outs=[cc2o.ap().opt()])
                gs2 = p3.tile([1, 2 * F], dt.float32)
                nc.scalar.dma_start(gs2[:], cc2o.ap())

                mu2 = p3.tile([1, F], dt.float32)
                nc.vector.tensor_scalar(mu2[:], gs2[:, 0:F], inv_n, None,
                                        Alu.mult)
                ve2 = p3.tile([1, F], dt.float32)
                ms2 = p3.tile([1, F], dt.float32)
                nc.vector.tensor_tensor(ms2[:], mu2[:], mu2[:], Alu.mult)
                nc.vector.tensor_scalar(ve2[:], gs2[:, F:2 * F], inv_n, None,
                                        Alu.mult)
                nc.vector.tensor_tensor(ve2[:], ve2[:], ms2[:], Alu.subtract)
                nc.vector.tensor_scalar(ve2[:], ve2[:], EPS, None, Alu.add)
                sd2 = p3.tile([1, F], dt.float32)
                nc.scalar.sqrt(sd2[:], ve2[:])
                is2 = p3.tile([1, F], dt.float32)
                nc.vector.reciprocal(is2[:], sd2[:])
                sc2 = p3.tile([1, F], dt.float32)
                nc.vector.tensor_tensor(sc2[:], gbn_sb[:], is2[:], Alu.mult)
                sh2 = p3.tile([1, F], dt.float32)
                nc.vector.tensor_tensor(sh2[:], mu2[:], sc2[:], Alu.mult)
                nc.vector.tensor_tensor(sh2[:], bbn_sb[:], sh2[:],
                                        Alu.subtract)
                b2ps = ps3.tile([P, F], dt.float32, tag="b2ps")
                nc.tensor.matmul(b2ps[:], ones1[:], sc2[:], start=True,
                                 stop=True)
                sv2 = p3.tile([P, F], dt.float32)
                nc.vector.tensor_copy(sv2[:], b2ps[:])
                b2ps2 = ps3.tile([P, F], dt.float32, tag="b2ps")
                nc.tensor.matmul(b2ps2[:], ones1[:], sh2[:], start=True,
                                 stop=True)
                bv2 = p3.tile([P, F], dt.float32)
                nc.vector.tensor_copy(bv2[:], b2ps2[:])

                # out = softplus(nf + bn(agg)), all row-major
                nfr = p3.tile([P, NGRP, F], dt.float32)
                nc.sync.dma_start(
                    nfr[:], nftr.ap().rearrange("p (g f) -> p g f", f=F))
                nc.vector.tensor_tensor(
                    aggm[:], aggm[:],
                    sv2[:, None, :].broadcast_to([P, NGRP, F]), Alu.mult)
                nc.vector.tensor_tensor(
                    aggm[:], aggm[:],
                    bv2[:, None, :].broadcast_to([P, NGRP, F]), Alu.add)
                nc.vector.tensor_tensor(aggm[:], aggm[:], nfr[:], Alu.add)
                u3 = p3.tile([P, NGRP, F], dt.float32)
                nc.scalar.activation(u3[:], aggm[:], Act.Exp)
                nc.scalar.activation(aggm[:], u3[:], Act.Ln, bias=1.0,
                                     scale=1.0)
                nc.sync.dma_start(
                    orow.ap().rearrange("p (g f) -> p g f", f=F), aggm[:])

    nc.compile()
    return nc


_CACHE = {}


def _prep(inputs, T=2048):
    nf = np.ascontiguousarray(np.asarray(inputs["node_feats"], np.float32))
    ef = np.ascontiguousarray(np.asarray(inputs["edge_feats"], np.float32))
    src = np.asarray(inputs["src"], np.int64)
    dst = np.asarray(inputs["dst"], np.int64)
    Wi = np.asarray(inputs["W_int"], np.float32)
    Wu = np.asarray(inputs["W_upd"], np.float32)
    N, Fn = nf.shape
    E, FE = ef.shape
    assert Fn == F
    cfg = _cfg(N, E, FE, T=T)
    NCh, CH, R, NCc = cfg["NCHUNK"], cfg["CH"], cfg["R"], cfg["NC"]

    # b_int/b_upd are dropped: a constant bias shifts mean equally and
    # cancels inside BatchNorm.
    Psrc = (nf @ np.concatenate([Wi[:F], Wu[:F]], axis=1)).astype(BF16)
    Pdst = (nf @ np.concatenate([Wi[F:2 * F], Wu[F:2 * F]], axis=1)).astype(BF16)
    W3 = np.concatenate([Wi[2 * F:], Wu[2 * F:]], axis=1).astype(BF16)

    core = dst // R
    chunk = src // CH
    key = core * NCh + chunk
    order = np.lexsort((src, key))
    counts = np.bincount(key, minlength=NCc * NCh)
    gstart = np.zeros(NCc * NCh + 1, np.int64)
    np.cumsum(counts, out=gstart[1:])

    # ---- occurrence-rank block filling -------------------------------
    # dma_scatter_add cannot accumulate duplicate indices within one call
    # (the CCE read-modify-write races between M2S reads and S2M writes),
    # so each block of B edges is split into rank segments: seg r holds
    # the (r+1)-th occurrences of dst values within the block, each seg
    # internally dst-unique, scattered by its own call into its own agg
    # buffer. Calls on one buffer are WAW-serialized by Tile.
    B = 2 * T

    def occ_ranks(d):
        o = np.argsort(d, kind="stable")
        sd = d[o]
        newrun = np.r_[True, sd[1:] != sd[:-1]]
        ii = np.arange(len(d))
        runstart = np.maximum.accumulate(np.where(newrun, ii, 0))
        occ = np.empty(len(d), np.int64)
        occ[o] = ii - runstart
        return occ

    prof = np.zeros(256, np.float64)
    npool = 0
    for g in range(NCc * NCh):
        dd = dst[order[gstart[g]:gstart[g + 1]]]
        for p0 in range(0, len(dd), B):
            oc = occ_ranks(dd[p0:p0 + B])
            bc = np.bincount(oc, minlength=256)[:256]
            prof += bc
            npool += 1
    prof /= max(npool, 1)
    segs = []
    for r in range(1, 256):
        if prof[r] < 24:
            break
        s_r = max(128, int(round(prof[r] / 128)) * 128)
        if sum(segs) + s_r > B - 512:
            break
        segs.append(s_r)
    SEGS = [B - sum(segs)] + segs
    cfg["SEGS"] = tuple(SEGS)
    soff = np.cumsum([0] + SEGS)

    def fill_chunk(eidx):
        blocks = []
        carry = np.empty(0, np.int64)
        ptr = 0
        n = len(eidx)
        while ptr < n or len(carry):
            take = min(B - len(carry), n - ptr)
            pool = np.concatenate([carry, eidx[ptr:ptr + take]])
            ptr += take
            oc = occ_ranks(dst[pool])
            slots = np.full(B, -1, np.int64)
            used = np.zeros(len(pool), bool)
            for r, sr in enumerate(SEGS):
                cand = np.flatnonzero(oc == r)[:sr]
                slots[soff[r]:soff[r] + len(cand)] = pool[cand]
                used[cand] = True
            carry = pool[~used]
            blocks.append(slots)
        return blocks

    core_blocks = []
    nbc = 0
    for c in range(NCc):
        per_chunk = []
        for k in range(NCh):
            g = c * NCh + k
            blks = fill_chunk(order[gstart[g]:gstart[g + 1]])
            nbc = max(nbc, len(blks))
            per_chunk.append(blks)
        core_blocks.append(per_chunk)

    tpc = 2 * nbc
    KT = tpc * T
    ETOT = NCh * KT
    cfg["TPC"], cfg["ETOT"] = tpc, ETOT

    in_maps = []
    psrc_arrs = []
    for k in range(NCh):
        tab = np.zeros((CH + 1, P), BF16)
        hi = min((k + 1) * CH, N)
        tab[: hi - k * CH] = Psrc[k * CH: hi]
        psrc_arrs.append(tab)
    gvec = np.concatenate([np.asarray(inputs["g_int"], np.float32),
                           np.asarray(inputs["g_upd"], np.float32)])[None, :]
    bvec = np.concatenate([np.asarray(inputs["be_int"], np.float32),
                           np.asarray(inputs["be_upd"], np.float32)])[None, :]
    gbn = np.asarray(inputs["g_bn"], np.float32)[None, :]
    bbn = np.asarray(inputs["be_bn"], np.float32)[None, :]

    for c in range(NCc):
        src_l = np.full(ETOT, CH, np.int16)
        dst_l = np.full(ETOT, R, np.int16)
        eftc = np.zeros((FE, ETOT), BF16)
        for k in range(NCh):
            slotc = np.full(KT, -1, np.int64)
            blks = core_blocks[c][k]
            for bi, blk in enumerate(blks):
                slotc[bi * B:(bi + 1) * B] = blk
            mask = slotc >= 0
            sel = slotc[mask]
            pos = np.flatnonzero(mask) + k * KT
            src_l[pos] = (src[sel] - k * CH).astype(np.int16)
            dst_l[pos] = (dst[sel] - c * R).astype(np.int16)
            eftc[:, pos] = ef[sel].T
        pd = np.zeros((cfg["R_PAD"], P), BF16)
        pd[:R] = Pdst[c * R:(c + 1) * R]
        nfp = np.zeros((cfg["R_PAD"], F), np.float32)
        nfp[:R] = nf[c * R:(c + 1) * R]
        ngrp = cfg["R_PAD"] // P
        nftc = np.ascontiguousarray(
            nfp.reshape(ngrp, P, F).transpose(1, 0, 2).reshape(P, ngrp * F))
        m = {
            "pdst": pd,
            "eft": eftc,
            "srcidx": np.ascontiguousarray(
                np.tile(src_l.reshape(ETOT // 16, 16).T, (P // 16, 1))),
            "dstidx": np.ascontiguousarray(
                np.tile(dst_l.reshape(ETOT // 16, 16).T, (P // 16, 1))),
            "nftr": nftc,
            "w3": W3,
            "gvr": gvec, "bvr": bvec, "gbn": gbn, "bbn": bbn,
        }
        for k in range(NCh):
            m[f"psrc{k}"] = psrc_arrs[k]
        in_maps.append(m)
    return cfg, in_maps


def _run(inputs, T=2048, trace=False):
    cfg, in_maps = _prep(inputs, T=T)
    ck = (cfg["N"], cfg["E"], cfg["FE"], cfg["T"], cfg["TPC"], cfg["SEGS"])
    if ck not in _CACHE:
        _CACHE[ck] = build_graph(cfg)
    nc = _CACHE[ck]
    res = run_bass_kernel_spmd(nc, in_maps, core_ids=list(range(cfg["NC"])),
                               trace=trace)
    R = cfg["R"]
    ngrp = cfg["R_PAD"] // P
    outs = []
    for c in range(cfg["NC"]):
        o = np.asarray(res.results[c]["orow"]).reshape(P, ngrp, F)
        outs.append(o.transpose(1, 0, 2).reshape(ngrp * P, F)[:R])
    out = np.concatenate(outs, axis=0)
    return np.ascontiguousarray(out, dtype=np.float32), res


def kernel(**inputs) -> np.ndarray:
    out, _ = _run(inputs)
    return out


# revision 22
# speedup vs baseline: 1.0628x; 1.0458x over previous
"""CGCNN conv kernel for 8 TRN2 NeuronCores (Bass/Tile).

Strategy (edge-parallel, dst-sharded, row-major):
  z @ W = nf[src] @ W[0:64] + nf[dst] @ W[64:128] + ef @ W[128:160]
  - Host precomputes P_src = nf @ [Wi[:64]|Wu[:64]]  (bf16 [N,128], 256B rows)
                    P_dst = nf @ [Wi[64:128]|Wu[64:128]]
  - Edges sorted by (dst//R, src//CH, src); core c owns dst range
    [c*R,(c+1)*R) so the segment-sum is core-local; src chunks of CH=25000
    keep gather indices in int16.
  - Pass 1 (row-major): non-transposed dma_gather of P rows (512-idx calls
    rotated over 4 SWDGE queues); PE computes ef@W3 into row-major PSUM;
    DVE adds the gathered rows; per-feature BN stats via strided reduces
    accumulated in [128,128] partials; x stored to DRAM bf16 row-major.
  - AllReduce [1,256] edge-BN stats; scale/shift broadcast to [128,128]
    tiles via PE ones-outer-product.
  - Pass 2: reload x; BN applied with broadcast_to DVE ops; ACT sigmoid /
    exp / ln(1+u); msg = gate*sp row-major feeds dma_scatter_add directly
    (occurrence-rank segments into rotating agg buffers, queue-rotated).
  - Phase 3: node-BN stats AllReduce [F,2]; out = softplus(nf + bn(agg))
    feat-major; host transposes back.
"""

import itertools
import math
import sys

import numpy as np

for _p in ("/opt/trn_rl_repo", "/root/.axon_site/_ro/trn_rl_repo"):
    if _p not in sys.path:
        sys.path.append(_p)

import ml_dtypes
from concourse import bacc, bass, mybir
from concourse import tile as ctile
from concourse.bass_utils import run_bass_kernel_spmd
from concourse.masks import make_identity

P = 128
F = 64  # node feature dim; 2F == P
EPS = 1e-5
BF16 = ml_dtypes.bfloat16
NQ = 4  # SWDGE queues
GQ = 1024  # indices per gather call

Alu = mybir.AluOpType
Act = mybir.ActivationFunctionType
dt = mybir.dt


def _cfg(N, E, FE, T=2048, ncores=8):
    R = N // ncores
    assert R * ncores == N
    nchunk = max(1, math.ceil(N / 25000))
    CH = math.ceil(N / nchunk)
    assert CH + 1 <= 32768 and R + 1 <= 32768
    r_pad = math.ceil((R + 1) / P) * P
    return dict(
        N=N, E=E, FE=FE, T=T, NC=ncores,
        R=R, NCHUNK=nchunk, CH=CH, R_PAD=r_pad,
    )


def build_graph(cfg):
    NC, T, FE = cfg["NC"], cfg["T"], cfg["FE"]
    CH, NCHUNK, R_PAD = cfg["CH"], cfg["NCHUNK"], cfg["R_PAD"]
    TPC, ETOT = cfg["TPC"], cfg["ETOT"]
    SEGS = list(cfg["SEGS"])
    nseg = len(SEGS)
    NAGG = 8  # rotating scatter-accumulator buffers
    NTILES = NCHUNK * TPC
    NBLK = NTILES // 2
    NGRP = R_PAD // P
    G = T // P  # row-major groups per tile
    B = 2 * T
    inv_e = 1.0 / float(cfg["E"])
    inv_n = 1.0 / float(cfg["N"])
    qc = itertools.count()
    sb = itertools.count()

    nc = bacc.Bacc("TRN2", target_bir_lowering=False, debug=False,
                   num_devices=NC, num_swdge_queues=NQ)

    psrc = [nc.dram_tensor(f"psrc{c}", [CH + 1, P], dt.bfloat16,
                           kind="ExternalInput") for c in range(NCHUNK)]
    pdst = nc.dram_tensor("pdst", [R_PAD, P], dt.bfloat16, kind="ExternalInput")
    eft = nc.dram_tensor("eft", [FE, ETOT], dt.bfloat16, kind="ExternalInput")
    srcidx = nc.dram_tensor("srcidx", [P, ETOT // 16], dt.int16,
                            kind="ExternalInput")
    dstidx = nc.dram_tensor("dstidx", [P, ETOT // 16], dt.int16,
                            kind="ExternalInput")
    nftr = nc.dram_tensor("nftr", [P, (R_PAD // P) * F], dt.float32,
                          kind="ExternalInput")
    w3 = nc.dram_tensor("w3", [FE, P], dt.bfloat16, kind="ExternalInput")
    gvr = nc.dram_tensor("gvr", [1, P], dt.float32, kind="ExternalInput")
    bvr = nc.dram_tensor("bvr", [1, P], dt.float32, kind="ExternalInput")
    gbn = nc.dram_tensor("gbn", [1, F], dt.float32, kind="ExternalInput")
    bbn = nc.dram_tensor("bbn", [1, F], dt.float32, kind="ExternalInput")
    orow = nc.dram_tensor("orow", [P, (R_PAD // P) * F], dt.float32,
                          kind="ExternalOutput")

    xrow = nc.dram_tensor("xrow", [NBLK, P, B], dt.bfloat16, kind="Internal")
    aggd = [nc.dram_tensor(f"aggd{r}", [NGRP, P, F], dt.float32,
                           kind="Internal") for r in range(NAGG)]
    cc1i = nc.dram_tensor("cc1i", [1, 2 * P], dt.float32, kind="Internal")
    cc1o = nc.dram_tensor("cc1o", [1, 2 * P], dt.float32, kind="Internal",
                          addr_space="Shared")
    cc2i = nc.dram_tensor("cc2i", [1, 2 * F], dt.float32, kind="Internal")
    cc2o = nc.dram_tensor("cc2o", [1, 2 * F], dt.float32, kind="Internal",
                          addr_space="Shared")
    rg = [list(range(NC))]

    with ctile.TileContext(nc) as tc:
        with tc.tile_pool(name="const", bufs=1) as cp:
            w3_sb = cp.tile([FE, P], dt.bfloat16)
            nc.sync.dma_start(w3_sb[:], w3.ap())
            gv = cp.tile([1, P], dt.float32)
            nc.sync.dma_start(gv[:], gvr.ap())
            bv = cp.tile([1, P], dt.float32)
            nc.sync.dma_start(bv[:], bvr.ap())
            gbn_sb = cp.tile([1, F], dt.float32)
            nc.sync.dma_start(gbn_sb[:], gbn.ap())
            bbn_sb = cp.tile([1, F], dt.float32)
            nc.sync.dma_start(bbn_sb[:], bbn.ap())
            ones1 = cp.tile([1, P], dt.float32)
            nc.vector.memset(ones1[:], 1.0)
            onesc = cp.tile([P, 1], dt.float32)
            nc.vector.memset(onesc[:], 1.0)

            accs = cp.tile([P, P], dt.float32)
            nc.vector.memset(accs[:], 0.0)
            accq = cp.tile([P, P], dt.float32)
            nc.vector.memset(accq[:], 0.0)
            svbc = cp.tile([P, P], dt.bfloat16)
            bvbc = cp.tile([P, P], dt.bfloat16)

            # zero-fill agg accumulators
            zb = cp.tile([P, 512], dt.float32)
            nc.vector.memset(zb[:], 0.0)
            gper = 512 // F
            for r in range(NAGG):
                for g0 in range(0, NGRP, gper):
                    ng = min(gper, NGRP - g0)
                    nc.sync.dma_start(aggd[r].ap()[g0:g0 + ng, :, :],
                                      zb[:, :ng * F])

            # ---------------- pass 1 (row-major) ----------------
            with tc.tile_pool(name="p1", bufs=6) as p1, \
                 tc.tile_pool(name="p1i", bufs=8) as p1i, \
                 tc.tile_pool(name="ps1", bufs=4, space="PSUM") as ps1:
                for c in range(NCHUNK):
                    for tl in range(TPC):
                        t = c * TPC + tl
                        sidx = p1i.tile([P, T // 16], dt.int16, tag="sidx")
                        nc.sync.dma_start(
                            sidx[:], srcidx.ap()[:, t * (T // 16):
                                                 (t + 1) * (T // 16)])
                        didx = p1i.tile([P, T // 16], dt.int16, tag="didx")
                        nc.sync.dma_start(
                            didx[:], dstidx.ap()[:, t * (T // 16):
                                                 (t + 1) * (T // 16)])
                        srcg = p1.tile([P, G, P], dt.bfloat16, tag="srcg")
                        dstg = p1.tile([P, G, P], dt.bfloat16, tag="dstg")
                        for q in range(T // GQ):
                            gs = slice(q * (GQ // P), (q + 1) * (GQ // P))
                            qi = slice(q * (GQ // 16), (q + 1) * (GQ // 16))
                            nc.gpsimd.dma_gather(
                                srcg[:, gs, :], psrc[c].ap(), sidx[:, qi],
                                GQ, GQ, P, queue_num=next(qc) % NQ)
                            nc.gpsimd.dma_gather(
                                dstg[:, gs, :], pdst.ap(), didx[:, qi],
                                GQ, GQ, P, queue_num=next(qc) % NQ)
                        eftt = p1.tile([FE, T], dt.bfloat16, tag="eftt")
                        nc.sync.dma_start(eftt[:], eft.ap()[:, t * T:(t + 1) * T])

                        xh = p1.tile([P, T], dt.bfloat16, tag="xh")
                        srcf = srcg[:].rearrange("p a b -> p (a b)")
                        dstf = dstg[:].rearrange("p a b -> p (a b)")
                        for s in range(T // 512):
                            ps = ps1.tile([P, 512], dt.float32, tag="ps")
                            for g in range(4):
                                col = (s * 4 + g) * P
                                nc.tensor.matmul(
                                    ps[:, g * P:(g + 1) * P],
                                    eftt[:, col:col + P], w3_sb[:],
                                    start=True, stop=True)
                            sl = slice(s * 512, (s + 1) * 512)
                            nc.vector.tensor_tensor(
                                xh[:, sl], ps[:], srcf[:, sl], Alu.add)
                            nc.vector.tensor_tensor(
                                xh[:, sl], xh[:, sl], dstf[:, sl], Alu.add)
                        nc.scalar.dma_start(
                            xrow.ap()[t // 2, :, (t % 2) * T:(t % 2 + 1) * T],
                            xh[:])
                        xsq = p1.tile([P, T], dt.bfloat16, tag="xsq")
                        nc.scalar.activation(xsq[:], xh[:], Act.Square)
                        # contiguous pairwise tree-fold of the 16 row-groups
                        # down to one [P,P] per-feature partial, then fp32
                        # accumulate
                        for tile, acc in ((xh, accs), (xsq, accq)):
                            for w in (8, 4, 2, 1):
                                nc.vector.tensor_tensor(
                                    tile[:, :w * P], tile[:, :w * P],
                                    tile[:, w * P:2 * w * P], Alu.add)
                            nc.vector.tensor_tensor(acc[:], acc[:],
                                                    tile[:, :P], Alu.add)

            # ---------------- edge-BN stats ----------------
            with tc.tile_pool(name="st", bufs=1) as stp, \
                 tc.tile_pool(name="pst", bufs=2, space="PSUM") as pst:
                sps = pst.tile([1, P], dt.float32, tag="sps")
                nc.tensor.matmul(sps[:], onesc[:], accs[:],
                                 start=True, stop=True)
                qps = pst.tile([1, P], dt.float32, tag="qps")
                nc.tensor.matmul(qps[:], onesc[:], accq[:],
                                 start=True, stop=True)
                cst = stp.tile([1, 2 * P], dt.float32)
                nc.vector.tensor_copy(cst[:, 0:P], sps[:])
                nc.vector.tensor_copy(cst[:, P:2 * P], qps[:])
                nc.sync.dma_start(cc1i.ap(), cst[:])
                nc.gpsimd.collective_compute(
                    "AllReduce", Alu.add, replica_groups=rg,
                    ins=[cc1i.ap().opt()], outs=[cc1o.ap().opt()])
                gst = stp.tile([1, 2 * P], dt.float32)
                nc.scalar.dma_start(gst[:], cc1o.ap())

                mu = stp.tile([1, P], dt.float32)
                nc.vector.tensor_scalar(mu[:], gst[:, 0:P], inv_e, None,
                                        Alu.mult)
                veps = stp.tile([1, P], dt.float32)
                musq = stp.tile([1, P], dt.float32)
                nc.vector.tensor_tensor(musq[:], mu[:], mu[:], Alu.mult)
                nc.vector.tensor_scalar(veps[:], gst[:, P:2 * P], inv_e, None,
                                        Alu.mult)
                nc.vector.tensor_tensor(veps[:], veps[:], musq[:],
                                        Alu.subtract)
                nc.vector.tensor_scalar(veps[:], veps[:], EPS, None, Alu.add)
                sdv = stp.tile([1, P], dt.float32)
                nc.scalar.sqrt(sdv[:], veps[:])
                isd = stp.tile([1, P], dt.float32)
                nc.vector.reciprocal(isd[:], sdv[:])
                scl = stp.tile([1, P], dt.float32)
                nc.vector.tensor_tensor(scl[:], gv[:], isd[:], Alu.mult)
                shf = stp.tile([1, P], dt.float32)
                nc.vector.tensor_tensor(shf[:], mu[:], scl[:], Alu.mult)
                nc.vector.tensor_tensor(shf[:], bv[:], shf[:], Alu.subtract)

                bps = pst.tile([P, P], dt.float32, tag="bps")
                nc.tensor.matmul(bps[:], ones1[:], scl[:], start=True,
                                 stop=True)
                nc.vector.tensor_copy(svbc[:], bps[:])
                bps2 = pst.tile([P, P], dt.float32, tag="bps")
                nc.tensor.matmul(bps2[:], ones1[:], shf[:], start=True,
                                 stop=True)
                nc.vector.tensor_copy(bvbc[:], bps2[:])

            # ---------------- pass 2 ----------------
            GB = B // P  # 32 row-groups per block
            soff = np.cumsum([0] + SEGS)
            PAIR = 2
            with tc.tile_pool(name="p2", bufs=2 * PAIR + 1) as p2, \
                 tc.tile_pool(name="p2i", bufs=6) as p2i:
                for b0 in range(0, NBLK, PAIR):
                    blks = range(b0, min(b0 + PAIR, NBLK))
                    xns, gates, us, sps, msgs = {}, {}, {}, {}, {}
                    for b in blks:
                        xi = p2.tile([P, GB, P], dt.bfloat16, tag="xi")
                        nc.sync.dma_start(
                            xi[:],
                            xrow.ap()[b].rearrange("p (a b) -> p a b", b=P))
                        xn = p2.tile([P, GB, P], dt.bfloat16, tag="xn")
                        nc.vector.tensor_tensor(
                            xn[:], xi[:],
                            svbc[:, None, :].broadcast_to([P, GB, P]),
                            Alu.mult)
                        nc.vector.tensor_tensor(
                            xn[:], xn[:],
                            bvbc[:, None, :].broadcast_to([P, GB, P]),
                            Alu.add)
                        xns[b] = xn
                    for b in blks:
                        gate = p2.tile([P, GB, F], dt.bfloat16, tag="gate")
                        nc.scalar.activation(gate[:], xns[b][:, :, 0:F],
                                             Act.Sigmoid)
                        gates[b] = gate
                    for b in blks:
                        u = p2.tile([P, GB, F], dt.bfloat16, tag="u")
                        nc.scalar.activation(u[:], xns[b][:, :, F:P], Act.Exp)
                        us[b] = u
                    for b in blks:
                        sp = p2.tile([P, GB, F], dt.bfloat16, tag="sp")
                        nc.scalar.activation(sp[:], us[b][:], Act.Ln,
                                             bias=1.0, scale=1.0)
                        sps[b] = sp
                    for b in blks:
                        msg = p2.tile([P, GB, F], dt.float32, tag="msg")
                        nc.vector.tensor_tensor(msg[:], gates[b][:],
                                                sps[b][:], Alu.mult)
                        msgs[b] = msg
                    for b in blks:
                        didx2 = p2i.tile([P, B // 16], dt.int16, tag="didx2")
                        nc.sync.dma_start(
                            didx2[:],
                            dstidx.ap()[:, b * (B // 16):(b + 1) * (B // 16)])
                        for r, sr in enumerate(SEGS):
                            ri = next(sb) % NAGG
                            o0 = int(soff[r])
                            nc.gpsimd.dma_scatter_add(
                                aggd[ri].ap().flatten_outer_dims(),
                                msgs[b][:, o0 // P:(o0 + sr) // P, :],
                                didx2[:, o0 // 16:(o0 + sr) // 16],
                                sr, sr, F, queue_num=next(qc) % NQ)

            # ---------------- phase 3 (row-major) ----------------
            with tc.tile_pool(name="p3", bufs=1) as p3, \
                 tc.tile_pool(name="p3c", bufs=4) as p3c, \
                 tc.tile_pool(name="ps3", bufs=1, space="PSUM") as ps3:
                Rr = cfg["R"]
                MG = 14  # node groups per merge chunk
                aggm = p3.tile([P, NGRP, F], dt.float32)
                for q0 in range(0, NGRP, MG):
                    nq_ = min(MG, NGRP - q0)
                    first = True
                    for r in range(NAGG):
                        at = p3c.tile([P, MG, F], dt.float32,
                                      tag=f"at{r % 4}")
                        nc.sync.dma_start(
                            at[:, :nq_, :],
                            aggd[r].ap()[q0:q0 + nq_].rearrange(
                                "g p d -> p g d"))
                        if first:
                            nc.vector.tensor_copy(
                                aggm[:, q0:q0 + nq_, :], at[:, :nq_, :])
                            first = False
                        else:
                            nc.vector.tensor_tensor(
                                aggm[:, q0:q0 + nq_, :],
                                aggm[:, q0:q0 + nq_, :], at[:, :nq_, :],
                                Alu.add)
                # scatter pads all target node row R; rows R+1.. stay
                # zero from the zero-fill. Zero row R via partition-offset
                # DMA (DVE cannot address partition 84).
                lastg = Rr // P
                p0pad = Rr - lastg * P
                assert lastg == NGRP - 1
                nc.sync.dma_start(aggm[p0pad:p0pad + 1, lastg, :],
                                  zb[0:1, :F])

                # node-BN stats: per-feature sums over [p, g] cells
                sacc = p3.tile([P, F], dt.float32)
                nc.vector.tensor_reduce(
                    sacc[:], aggm[:].rearrange("p g f -> p f g"),
                    mybir.AxisListType.X, Alu.add)
                sqm = p3.tile([P, NGRP, F], dt.bfloat16)
                nc.scalar.activation(sqm[:], aggm[:], Act.Square)
                qacc = p3.tile([P, F], dt.float32)
                nc.vector.tensor_reduce(
                    qacc[:], sqm[:].rearrange("p g f -> p f g"),
                    mybir.AxisListType.X, Alu.add)
                s2ps = ps3.tile([1, F], dt.float32, tag="s2ps")
                nc.tensor.matmul(s2ps[:], onesc[:], sacc[:], start=True,
                                 stop=True)
                q2ps = ps3.tile([1, F], dt.float32, tag="q2ps")
                nc.tensor.matmul(q2ps[:], onesc[:], qacc[:], start=True,
                                 stop=True)
                c2st = p3.tile([1, 2 * F], dt.float32)
                nc.vector.tensor_copy(c2st[:, 0:F], s2ps[:])
                nc.vector.tensor_copy(c2st[:, F:2 * F], q2ps[:])
                nc.sync.dma_start(cc2i.ap(), c2st[:])
                nc.gpsimd.collective_compute(
                    "AllReduce", Alu.add, replica_groups=rg,
                    ins=[cc2i.ap().opt()], outs=[cc2o.ap().opt()])
                gs2 = p3.tile([1, 2 * F], dt.float32)
                nc.scalar.dma_start(gs2[:], cc2o.ap())

                mu2 = p3.tile([1, F], dt.float32)
                nc.vector.tensor_scalar(mu2[:], gs2[:, 0:F], inv_n, None,
                                        Alu.mult)
                ve2 = p3.tile([1, F], dt.float32)
                ms2 = p3.tile([1, F], dt.float32)
                nc.vector.tensor_tensor(ms2[:], mu2[:], mu2[:], Alu.mult)
                nc.vector.tensor_scalar(ve2[:], gs2[:, F:2 * F], inv_n, None,
                                        Alu.mult)
                nc.vector.tensor_tensor(ve2[:], ve2[:], ms2[:], Alu.subtract)
                nc.vector.tensor_scalar(ve2[:], ve2[:], EPS, None, Alu.add)
                sd2 = p3.tile([1, F], dt.float32)
                nc.scalar.sqrt(sd2[:], ve2[:])
                is2 = p3.tile([1, F], dt.float32)
                nc.vector.reciprocal(is2[:], sd2[:])
                sc2 = p3.tile([1, F], dt.float32)
                nc.vector.tensor_tensor(sc2[:], gbn_sb[:], is2[:], Alu.mult)
                sh2 = p3.tile([1, F], dt.float32)
                nc.vector.tensor_tensor(sh2[:], mu2[:], sc2[:], Alu.mult)
                nc.vector.tensor_tensor(sh2[:], bbn_sb[:], sh2[:],
                                        Alu.subtract)
                b2ps = ps3.tile([P, F], dt.float32, tag="b2ps")
                nc.tensor.matmul(b2ps[:], ones1[:], sc2[:], start=True,
                                 stop=True)
                sv2 = p3.tile([P, F], dt.float32)
                nc.vector.tensor_copy(sv2[:], b2ps[:])
                b2ps2 = ps3.tile([P, F], dt.float32, tag="b2ps")
                nc.tensor.matmul(b2ps2[:], ones1[:], sh2[:], start=True,
                                 stop=True)
                bv2 = p3.tile([P, F], dt.float32)
                nc.vector.tensor_copy(bv2[:], b2ps2[:])

                # out = softplus(nf + bn(agg)), all row-major
                nfr = p3.tile([P, NGRP, F], dt.float32)
                nc.sync.dma_start(
                    nfr[:], nftr.ap().rearrange("p (g f) -> p g f", f=F))
                nc.vector.tensor_tensor(
                    aggm[:], aggm[:],
                    sv2[:, None, :].broadcast_to([P, NGRP, F]), Alu.mult)
                nc.vector.tensor_tensor(
                    aggm[:], aggm[:],
                    bv2[:, None, :].broadcast_to([P, NGRP, F]), Alu.add)
                nc.vector.tensor_tensor(aggm[:], aggm[:], nfr[:], Alu.add)
                u3 = p3.tile([P, NGRP, F], dt.float32)
                nc.scalar.activation(u3[:], aggm[:], Act.Exp)
                nc.scalar.activation(aggm[:], u3[:], Act.Ln, bias=1.0,
                                     scale=1.0)
                nc.sync.dma_start(
                    orow.ap().rearrange("p (g f) -> p g f", f=F), aggm[:])

    nc.compile()
    return nc


_CACHE = {}


def _prep(inputs, T=2048):
    nf = np.ascontiguousarray(np.asarray(inputs["node_feats"], np.float32))
    ef = np.ascontiguousarray(np.asarray(inputs["edge_feats"], np.float32))
    src = np.asarray(inputs["src"], np.int64)
    dst = np.asarray(inputs["dst"], np.int64)
    Wi = np.asarray(inputs["W_int"], np.float32)
    Wu = np.asarray(inputs["W_upd"], np.float32)
    N, Fn = nf.shape
    E, FE = ef.shape
    assert Fn == F
    cfg = _cfg(N, E, FE, T=T)
    NCh, CH, R, NCc = cfg["NCHUNK"], cfg["CH"], cfg["R"], cfg["NC"]

    # b_int/b_upd are dropped: a constant bias shifts mean equally and
    # cancels inside BatchNorm.
    Psrc = (nf @ np.concatenate([Wi[:F], Wu[:F]], axis=1)).astype(BF16)
    Pdst = (nf @ np.concatenate([Wi[F:2 * F], Wu[F:2 * F]], axis=1)).astype(BF16)
    W3 = np.concatenate([Wi[2 * F:], Wu[2 * F:]], axis=1).astype(BF16)

    core = dst // R
    chunk = src // CH
    key = core * NCh + chunk
    order = np.lexsort((src, key))
    counts = np.bincount(key, minlength=NCc * NCh)
    gstart = np.zeros(NCc * NCh + 1, np.int64)
    np.cumsum(counts, out=gstart[1:])

    # ---- occurrence-rank block filling -------------------------------
    # dma_scatter_add cannot accumulate duplicate indices within one call
    # (the CCE read-modify-write races between M2S reads and S2M writes),
    # so each block of B edges is split into rank segments: seg r holds
    # the (r+1)-th occurrences of dst values within the block, each seg
    # internally dst-unique, scattered by its own call into its own agg
    # buffer. Calls on one buffer are WAW-serialized by Tile.
    B = 2 * T

    def occ_ranks(d):
        o = np.argsort(d, kind="stable")
        sd = d[o]
        newrun = np.r_[True, sd[1:] != sd[:-1]]
        ii = np.arange(len(d))
        runstart = np.maximum.accumulate(np.where(newrun, ii, 0))
        occ = np.empty(len(d), np.int64)
        occ[o] = ii - runstart
        return occ

    prof = np.zeros(256, np.float64)
    npool = 0
    for g in range(NCc * NCh):
        dd = dst[order[gstart[g]:gstart[g + 1]]]
        for p0 in range(0, len(dd), B):
            oc = occ_ranks(dd[p0:p0 + B])
            bc = np.bincount(oc, minlength=256)[:256]
            prof += bc
            npool += 1
    prof /= max(npool, 1)
    segs = []
    for r in range(1, 256):
        if prof[r] < 24:
            break
        s_r = max(128, int(round(prof[r] / 128)) * 128)
        if sum(segs) + s_r > B - 512:
            break
        segs.append(s_r)
    SEGS = [B - sum(segs)] + segs
    cfg["SEGS"] = tuple(SEGS)
    soff = np.cumsum([0] + SEGS)

    def fill_chunk(eidx):
        blocks = []
        carry = np.empty(0, np.int64)
        ptr = 0
        n = len(eidx)
        while ptr < n or len(carry):
            take = min(B - len(carry), n - ptr)
            pool = np.concatenate([carry, eidx[ptr:ptr + take]])
            ptr += take
            oc = occ_ranks(dst[pool])
            slots = np.full(B, -1, np.int64)
            used = np.zeros(len(pool), bool)
            for r, sr in enumerate(SEGS):
                cand = np.flatnonzero(oc == r)[:sr]
                slots[soff[r]:soff[r] + len(cand)] = pool[cand]
                used[cand] = True
            carry = pool[~used]
            blocks.append(slots)
        return blocks

    core_blocks = []
    nbc = 0
    for c in range(NCc):
        per_chunk = []
        for k in range(NCh):
            g = c * NCh + k
            blks = fill_chunk(order[gstart[g]:gstart[g + 1]])
            nbc = max(nbc, len(blks))
            per_chunk.append(blks)
        core_blocks.append(per_chunk)

    tpc = 2 * nbc
    KT = tpc * T
    ETOT = NCh * KT
    cfg["TPC"], cfg["ETOT"] = tpc, ETOT

    in_maps = []
    psrc_arrs = []
    for k in range(NCh):
        tab = np.zeros((CH + 1, P), BF16)
        hi = min((k + 1) * CH, N)
        tab[: hi - k * CH] = Psrc[k * CH: hi]
        psrc_arrs.append(tab)
    gvec = np.concatenate([np.asarray(inputs["g_int"], np.float32),
                           np.asarray(inputs["g_upd"], np.float32)])[None, :]
    bvec = np.concatenate([np.asarray(inputs["be_int"], np.float32),
                           np.asarray(inputs["be_upd"], np.float32)])[None, :]
    gbn = np.asarray(inputs["g_bn"], np.float32)[None, :]
    bbn = np.asarray(inputs["be_bn"], np.float32)[None, :]

    for c in range(NCc):
        src_l = np.full(ETOT, CH, np.int16)
        dst_l = np.full(ETOT, R, np.int16)
        eftc = np.zeros((FE, ETOT), BF16)
        for k in range(NCh):
            slotc = np.full(KT, -1, np.int64)
            blks = core_blocks[c][k]
            for bi, blk in enumerate(blks):
                slotc[bi * B:(bi + 1) * B] = blk
            mask = slotc >= 0
            sel = slotc[mask]
            pos = np.flatnonzero(mask) + k * KT
            src_l[pos] = (src[sel] - k * CH).astype(np.int16)
            dst_l[pos] = (dst[sel] - c * R).astype(np.int16)
            eftc[:, pos] = ef[sel].T
        pd = np.zeros((cfg["R_PAD"], P), BF16)
        pd[:R] = Pdst[c * R:(c + 1) * R]
        nfp = np.zeros((cfg["R_PAD"], F), np.float32)
        nfp[:R] = nf[c * R:(c + 1) * R]
        ngrp = cfg["R_PAD"] // P
        nftc = np.ascontiguousarray(
            nfp.reshape(ngrp, P, F).transpose(1, 0, 2).reshape(P, ngrp * F))
        m = {
            "pdst": pd,
            "eft": eftc,
            "srcidx": np.ascontiguousarray(
                np.tile(src_l.reshape(ETOT // 16, 16).T, (P // 16, 1))),
            "dstidx": np.ascontiguousarray(
                np.tile(dst_l.reshape(ETOT // 16, 16).T, (P // 16, 1))),
            "nftr": nftc,
            "w3": W3,
            "gvr": gvec, "bvr": bvec, "gbn": gbn, "bbn": bbn,
        }
        for k in range(NCh):
            m[f"psrc{k}"] = psrc_arrs[k]
        in_maps.append(m)
    return cfg, in_maps


def _run(inputs, T=2048, trace=False):
    cfg, in_maps = _prep(inputs, T=T)
    ck = (cfg["N"], cfg["E"], cfg["FE"], cfg["T"], cfg["TPC"], cfg["SEGS"])
    if ck not in _CACHE:
        _CACHE[ck] = build_graph(cfg)
    nc = _CACHE[ck]
    res = run_bass_kernel_spmd(nc, in_maps, core_ids=list(range(cfg["NC"])),
                               trace=trace)
    R = cfg["R"]
    ngrp = cfg["R_PAD"] // P
    outs = []
    for c in range(cfg["NC"]):
        o = np.asarray(res.results[c]["orow"]).reshape(P, ngrp, F)
        outs.append(o.transpose(1, 0, 2).reshape(ngrp * P, F)[:R])
    out = np.concatenate(outs, axis=0)
    return np.ascontiguousarray(out, dtype=np.float32), res


def kernel(**inputs) -> np.ndarray:
    out, _ = _run(inputs)
    return out
